# revision 12
# baseline (speedup 1.0000x reference)
"""Trainium2 Bass kernel for nn_Attention_Block (dense transformer block).

Strategy: pure data-parallel over batch — 8 samples, 8 NeuronCores, one
sample per core, weights replicated, no collectives. Per core everything
stays channels-on-partitions (c x n layout).

v2: fp8(e4m3) DoubleRow matmuls for all dense GEMMs (QKV, out-proj,
SwiGLU MLP1/MLP2) at ~2x PE rate; attention QK as row-tiled concurrent
matmul pairs (d=64 contraction, two heads in disjoint PE row groups);
AV in fp8 DoubleRow with the softmax denominator riding as a ones
column of V^T; exp emitted as fp8 directly from the activation engine.

  GN1 (bn_stats + selector-matmul group reduce) -> QKV (fp8 DR) ->
  per-head attention (row-tiled K^T Q, exp->fp8, fp8-DR AV+denominator,
  transpose/reciprocal/selector-broadcast normalize) -> out-proj (fp8
  DR) -> GN2 -> SwiGLU MLP (fp8 DR) -> +residual.
"""

import os

import numpy as np
import ml_dtypes

KSTAGE = int(os.environ.get("KSTAGE", "7"))

C = 512
NSP = 1024  # 32*32 spatial
CT = 4  # channel tiles of 128
HEADS = 8
D = 64
HID = 2048
EPS = 1e-5

_cache = {}


def _patch_tile_drain(tile, mybir):
    """walrus in this environment accepts very few sync waits per
    instruction; the TileContext tail drain carries one wait per proc of
    the global clock. Split them across preceding SP drains."""
    if getattr(tile.TileContext, "_drain_patched", False):
        return

    def _patched(self, tick_clock, wait_clock):
        nc = self.nc
        spills = [nc.sync.drain() for _ in range(40)]
        drain_inst = nc.sync.drain()
        wait_clock.add_sem_waits(
            drain_inst.ins, tile.ScopedClock({None: tick_clock.global_clock})
        )
        si = drain_inst.ins.sync_info
        waits = list(si.on_wait) if si is not None and si.on_wait else []
        upds = list(si.on_update) if si is not None and si.on_update else []
        if len(waits) > 1:
            *pre, last = waits
            assert len(pre) <= len(spills), "too many drain wait chunks"
            for sp_inst, w in zip(spills, pre):
                sp_inst.ins.sync_info = mybir.SyncInfo(on_wait=[w], on_update=[])
            drain_inst.ins.sync_info = mybir.SyncInfo(on_wait=[last], on_update=upds)
        nc.all_engine_barrier()
        assert self.sems is not None
        popped = nc._tile_sem_poison_stack.pop()
        assert popped is self._sem_poison
        nc.clear_and_free_semaphores(list(self.sems.allocated().values()))
        nc.all_engine_barrier()

    tile.TileContext._drain_and_barrier = _patched
    tile.TileContext._drain_patched = True


def _split_multi_waits(nc, mybir, maxw=1):
    """Hoist extra sync waits onto same-engine EventSemaphore carriers so
    no instruction carries more than `maxw` waits."""
    f = nc.m.functions[0]
    for bb in f.blocks:
        insts = list(bb.instructions)
        need = [
            i
            for i in insts
            if getattr(i, "sync_info", None)
            and i.sync_info.on_wait
            and len(i.sync_info.on_wait) > maxw
        ]
        if not need:
            continue
        carriers = {}
        for inst in need:
            w = list(inst.sync_info.on_wait)
            upds = list(inst.sync_info.on_update) if inst.sync_info.on_update else []
            keep = w[-maxw:]
            extra = w[:-maxw]
            cs = []
            for i in range(0, len(extra), maxw):
                c = mybir.InstEventSemaphore(
                    name=f"I-waitc-{nc.next_id()}", ins=[], outs=[]
                )
                c.engine = inst.engine
                c.sync_info = mybir.SyncInfo(on_wait=extra[i : i + maxw], on_update=[])
                nc.register_instruction(c)
                cs.append(c)
            inst.sync_info = mybir.SyncInfo(on_wait=keep, on_update=upds)
            carriers[inst.name] = cs
        carrier_names = {c.name for cs in carriers.values() for c in cs}
        rebuilt = []
        for inst in list(bb.instructions):
            if inst.name in carrier_names:
                continue
            if inst.name in carriers:
                rebuilt.extend(carriers[inst.name])
            rebuilt.append(inst)
        bb.instructions = rebuilt


def _build_nc():
    import concourse.bass as bass
    import concourse.tile as tile
    from concourse import mybir

    _patch_tile_drain(tile, mybir)

    F32 = mybir.dt.float32
    BF16 = mybir.dt.bfloat16
    FP8 = mybir.dt.float8e4
    ADD = mybir.AluOpType.add
    SUB = mybir.AluOpType.subtract
    MULT = mybir.AluOpType.mult
    AF = mybir.ActivationFunctionType
    DR = mybir.MatmulPerfMode.DoubleRow

    nc = bass.Bass()

    x_d = nc.declare_dram_parameter("x", [C, NSP], F32, isOutput=False)
    wqkv_d = nc.declare_dram_parameter("wqkv8", [128, 2, 2, 3 * C], FP8, isOutput=False)
    qkvb_d = nc.declare_dram_parameter("qkvb", [128, 12], F32, isOutput=False)
    wo_d = nc.declare_dram_parameter("wo8", [128, 2, 2, C], FP8, isOutput=False)
    outb_d = nc.declare_dram_parameter("outb", [128, 4], F32, isOutput=False)
    g1_d = nc.declare_dram_parameter("g1", [128, 4], F32, isOutput=False)
    b1_d = nc.declare_dram_parameter("b1", [128, 4], F32, isOutput=False)
    g2_d = nc.declare_dram_parameter("g2", [128, 4], F32, isOutput=False)
    b2_d = nc.declare_dram_parameter("b2", [128, 4], F32, isOutput=False)
    w1_d = nc.declare_dram_parameter("w18", [128, 2, 2, 2 * HID], FP8, isOutput=False)
    w2_d = nc.declare_dram_parameter("w28", [128, 8, 2, C], FP8, isOutput=False)
    sel_d = nc.declare_dram_parameter("sel", [C, 32], F32, isOutput=False)
    selT_d = nc.declare_dram_parameter("selT", [32, C], F32, isOutput=False)
    id_d = nc.declare_dram_parameter("ident", [128, 128], BF16, isOutput=False)
    selbc_d = nc.declare_dram_parameter("selbc", [16, 1024], BF16, isOutput=False)
    out_d = nc.declare_dram_parameter("out", [C, NSP], F32, isOutput=True)

    with tile.TileContext(nc) as tc:
        with (
            tc.tile_pool(name="pers", bufs=1) as pers,
            tc.tile_pool(name="gnp", bufs=2) as gnp,
            tc.tile_pool(name="swp", bufs=2) as swp,
            tc.tile_pool(name="invp", bufs=2) as invp,
            tc.tile_pool(name="ps", bufs=2, space="PSUM") as ps_pool,
            tc.tile_pool(name="psav", bufs=1, space="PSUM") as psav_pool,
            tc.tile_pool(name="psdn", bufs=2, space="PSUM") as psdn_pool,
        ):
            def pstile(shape, dtype):
                return ps_pool.tile(shape, dtype, tag="ps", name="ps")

            def pstile_av(shape, dtype, side):
                return psav_pool.tile(shape, dtype, tag=f"pav{side}", name="pav")

            def pstile_dn(shape, dtype):
                return psdn_pool.tile(shape, dtype, tag="dns", name="dns")

            # ---- input loads ----
            x_sb = []
            for t in range(CT):
                xt = pers.tile([128, NSP], F32, tag=f"x{t}", name=f"x{t}")
                nc.sync.dma_start(xt[:], x_d[t * 128 : (t + 1) * 128, :])
                x_sb.append(xt)
            sel_sb = []
            for t in range(CT):
                st = pers.tile([128, 32], F32, tag=f"sel{t}", name=f"sel{t}")
                nc.sync.dma_start(st[:], sel_d[t * 128 : (t + 1) * 128, :])
                sel_sb.append(st)
            selT_sb = pers.tile([32, C], F32, tag="selT", name="selT")
            nc.sync.dma_start(selT_sb[:], selT_d[:, :])
            g1_sb = pers.tile([128, 4], F32, tag="g1", name="g1")
            nc.sync.dma_start(g1_sb[:], g1_d[:, :])
            b1_sb = pers.tile([128, 4], F32, tag="b1", name="b1")
            nc.sync.dma_start(b1_sb[:], b1_d[:, :])
            wqkv_sb = pers.tile([128, 2, 2, 3 * C], FP8, tag="wqkv", name="wqkv")
            nc.sync.dma_start(wqkv_sb[:], wqkv_d[:, :, :, :])
            qkvb_sb = pers.tile([128, 12], F32, tag="qkvb", name="qkvb")
            nc.sync.dma_start(qkvb_sb[:], qkvb_d[:, :])
            selbc_sb = pers.tile([16, 1024], BF16, tag="selbc", name="selbc")
            nc.sync.dma_start(selbc_sb[:], selbc_d[:, :])
            id_sb = pers.tile([128, 128], BF16, tag="ident", name="ident")
            nc.sync.dma_start(id_sb[:], id_d[:, :])
            wo_sb = pers.tile([128, 2, 2, C], FP8, tag="wo", name="wo")
            nc.sync.dma_start(wo_sb[:], wo_d[:, :, :, :])
            outb_sb = pers.tile([128, 4], F32, tag="outb", name="outb")
            nc.sync.dma_start(outb_sb[:], outb_d[:, :])
            g2_sb = pers.tile([128, 4], F32, tag="g2", name="g2")
            nc.sync.dma_start(g2_sb[:], g2_d[:, :])
            b2_sb = pers.tile([128, 4], F32, tag="b2", name="b2")
            nc.sync.dma_start(b2_sb[:], b2_d[:, :])
            w1_sb = pers.tile([128, 2, 2, 2 * HID], FP8, tag="w1", name="w1")
            nc.sync.dma_start(w1_sb[:], w1_d[:, :, :, :])
            w2_sb = pers.tile([128, 8, 2, C], FP8, tag="w2", name="w2")
            nc.sync.dma_start(w2_sb[:], w2_d[:, :, :, :])

            eps32 = pers.tile([32, 1], F32, tag="eps", name="eps")
            nc.vector.memset(eps32[:], EPS)
            ones65 = pers.tile([65, 1], F32, tag="ones65", name="ones65")
            nc.vector.memset(ones65[:], 1.0)
            bias2 = pers.tile([128, 1], F32, tag="bias2", name="bias2")
            nc.vector.memset(bias2[:], 2.0)

            # ---- group norm helper (32 groups of 16 channels x 1024) ----
            def gn_stats(src_tile, t):
                stats = gnp.tile([128, 2, 6], F32, tag="gn_stats", name="gn_stats")
                for j2 in range(2):
                    nc.vector.bn_stats(
                        stats[:, j2, :], src_tile[:, j2 * 512 : (j2 + 1) * 512]
                    )
                mv = gnp.tile([128, 2], F32, tag="gn_mv", name="gn_mv")
                nc.vector.bn_aggr(mv[:], stats[:])
                r3 = gnp.tile([128, 3], F32, tag=f"gn_r3_{t}", name=f"gn_r3_{t}")
                nc.vector.tensor_copy(r3[:, 0:2], mv[:])
                nc.vector.tensor_mul(r3[:, 2:3], mv[:, 0:1], mv[:, 0:1])
                return r3

            def gn_finish(rhs3, src_tiles, gam_sb, bet_sb, dst8):
                pg = pstile([32, 3], F32)
                for t in range(CT):
                    nc.tensor.matmul(
                        pg[:], sel_sb[t][:], rhs3[t][:], start=(t == 0), stop=(t == 3)
                    )
                gs = gnp.tile([32, 2], F32, tag="gn_gs", name="gn_gs")
                tmp = gnp.tile([32, 2], F32, tag="gn_tmp", name="gn_tmp")
                pgs = gnp.tile([32, 3], F32, tag="gn_pgs", name="gn_pgs")
                nc.vector.tensor_copy(pgs[:], pg[:])
                # mean_g, E[x^2]_g, var_g, rstd_g
                nc.vector.tensor_scalar_mul(gs[:, 0:1], pgs[:, 0:1], 1.0 / 16)
                nc.vector.tensor_tensor(tmp[:, 0:1], pgs[:, 1:2], pgs[:, 2:3], op=ADD)
                nc.vector.tensor_scalar_mul(tmp[:, 0:1], tmp[:, 0:1], 1.0 / 16)
                nc.vector.tensor_mul(tmp[:, 1:2], gs[:, 0:1], gs[:, 0:1])
                nc.vector.tensor_tensor(tmp[:, 0:1], tmp[:, 0:1], tmp[:, 1:2], op=SUB)
                nc.scalar.activation(
                    tmp[:, 0:1], tmp[:, 0:1], AF.Sqrt, bias=eps32[:]
                )
                nc.vector.reciprocal(gs[:, 1:2], tmp[:, 0:1])
                for t in range(CT):
                    pbc = pstile([128, 2], F32)
                    nc.tensor.matmul(
                        pbc[:],
                        selT_sb[:, t * 128 : (t + 1) * 128],
                        gs[:],
                        start=True,
                        stop=True,
                    )
                    a_t = gnp.tile([128, 1], F32, tag="gn_A", name="gn_A")
                    b_t = gnp.tile([128, 1], F32, tag="gn_B", name="gn_B")
                    nc.vector.tensor_mul(a_t[:], pbc[:, 1:2], gam_sb[:, t : t + 1])
                    nc.vector.tensor_mul(b_t[:], pbc[:, 0:1], a_t[:])
                    nc.vector.tensor_tensor(
                        b_t[:], bet_sb[:, t : t + 1], b_t[:], op=SUB
                    )
                    nc.vector.tensor_scalar(
                        dst8[:, t // 2, t % 2, :],
                        src_tiles[t][:],
                        scalar1=a_t[:],
                        scalar2=b_t[:],
                        op0=MULT,
                        op1=ADD,
                    )

            # ---- GN1 -> xn8 (fp8, DoubleRow rhs layout) ----
            xn8 = pers.tile([128, 2, 2, NSP], FP8, tag="xn8", name="xn8")
            gn_finish([gn_stats(x_sb[t], t) for t in range(CT)],
                      x_sb, g1_sb, b1_sb, xn8)

            def dump8(slices):
                # slices: list of 4 fp8 [128, NSP] APs -> f32 dram
                for t in range(CT):
                    ft = pers.tile([128, NSP], F32, tag=f"dump{t}", name=f"dump{t}")
                    nc.vector.tensor_copy(ft[:], slices[t])
                    nc.sync.dma_start(out_d[t * 128 : (t + 1) * 128, :], ft[:])

            if KSTAGE == 1:
                dump8([xn8[:, t // 2, t % 2, :] for t in range(CT)])
                return nc

            # ---- QKV (12 out tiles of 128 x 1024; q/k bf16, v bf16) ----
            qk_sb = [
                pers.tile([128, NSP], BF16, tag=f"qk{m}", name=f"qk{m}")
                for m in range(8)
            ]
            v_sb = [
                pers.tile([128, NSP], BF16, tag=f"v{m}", name=f"v{m}")
                for m in range(4)
            ]
            for m in range(12):
                ps = pstile([128, NSP], F32)
                for n2 in range(2):
                    s = slice(n2 * 512, (n2 + 1) * 512)
                    for t in range(2):
                        nc.tensor.matmul(
                            ps[:, s],
                            wqkv_sb[:, t, :, m * 128 : (m + 1) * 128],
                            xn8[:, t, :, s],
                            start=(t == 0),
                            stop=(t == 1),
                            perf_mode=DR,
                        )
                dst = qk_sb[m] if m < 8 else v_sb[m - 8]
                nc.vector.tensor_scalar_add(dst[:], ps[:], qkvb_sb[:, m : m + 1])

            if KSTAGE == 2:
                dump8([qk_sb[t][:] for t in range(CT)])
                return nc

            # ---- attention (head pairs j: even head rows 0:64, odd 64:128) ----
            # vt2 layout per (pair t, sub i): [0:64]=V_even^T, [64]=ones,
            # [80:144]=V_odd^T, [144]=ones  (i-stride 176 B, %16 == 0)
            vts = []
            for vi in range(4):
                vt = pers.tile([128, 4, 2, 176], FP8, tag=f"vt{vi}", name=f"vt{vi}")
                nc.vector.memset(vt[:, :, :, 64:65], 1.0)
                nc.vector.memset(vt[:, :, :, 144:145], 1.0)
                vts.append(vt)
            # e8 buffers per (parity, side)
            e8 = [
                [
                    pers.tile(
                        [128, 4, 2, NSP], FP8, tag=f"e8_{p}{s}", name=f"e8_{p}{s}"
                    )
                    for s in range(2)
                ]
                for p in range(2)
            ]
            un_bf = [
                pers.tile([65, NSP], F32, tag=f"un{r}", name=f"un{r}")
                for r in range(8)
            ]
            xatt8 = pers.tile([128, 2, 2, NSP], FP8, tag="xatt8", name="xatt8")

            def build_vt(j):
                # V^T via PE transposes (bf16) -> fp8 slices of vt
                vt = vts[j]
                for mk in range(8):
                    pv = pstile_dn([128, 128], BF16)
                    nc.tensor.transpose(
                        pv[:], v_sb[j][:, mk * 128 : (mk + 1) * 128], id_sb[:]
                    )
                    t, i = mk // 2, mk % 2
                    nc.vector.tensor_copy(vt[:, t, i, 0:64], pv[:, 0:64])
                    nc.vector.tensor_copy(vt[:, t, i, 80:144], pv[:, 64:128])

            pav_cur = {}

            def av_step(j, side, nh, t):
                # one AV DoubleRow matmul (denominator rides as ones column);
                # interleaved between QK matmuls of iteration j+1.
                vt = vts[j]
                es = e8[j % 2]
                s = slice(nh * 512, (nh + 1) * 512)
                if t == 0:
                    pav_cur[side] = pstile_av([65, 512], F32, side)
                nc.tensor.matmul(
                    pav_cur[side][:, :],
                    vt[:, t, :, 80 * side : 80 * side + 65],
                    es[side][:, t, :, s],
                    start=(t == 0),
                    stop=(t == 3),
                    perf_mode=DR,
                )
                if t == 3:
                    nc.vector.tensor_copy(
                        un_bf[2 * j + side][:, s], pav_cur[side][0:65, :]
                    )

            av_plan = [
                (side, nh, t) for side in range(2) for nh in range(2)
                for t in range(4)
            ]

            def dance(j):
                # denominators: transpose the two denom rows into partitions,
                # one parallel reciprocal, selector-broadcast, normalize.
                pdt = pstile_dn([128, 16], F32)
                for side in range(2):
                    for jj in range(8):
                        nc.tensor.transpose(
                            pdt[:, side * 8 + jj : side * 8 + jj + 1],
                            un_bf[2 * j + side][64:65, jj * 128 : (jj + 1) * 128],
                            ones65[64:65, 0:1],
                        )
                inv16 = invp.tile([128, 16], F32, tag="inv", name="inv")
                nc.vector.reciprocal(inv16[:], pdt[:])
                inv16b = invp.tile([128, 16], BF16, tag="invb16", name="invb16")
                nc.vector.tensor_copy(inv16b[:], inv16[:])
                ptv = pstile_dn([16, 128], BF16)
                nc.tensor.transpose(ptv[:], inv16b[:], id_sb[:])
                pts = invp.tile([16, 128], BF16, tag="pts", name="pts")
                nc.vector.tensor_copy(pts[:], ptv[:])
                for side in range(2):
                    for nh in range(2):
                        s = slice(nh * 512, (nh + 1) * 512)
                        pinvb = pstile_dn([64, 512], F32)
                        for jj in range(4):
                            r = side * 8 + nh * 4 + jj
                            nc.tensor.matmul(
                                pinvb[:, jj * 128 : (jj + 1) * 128],
                                selbc_sb[:, r * 64 : (r + 1) * 64],
                                pts[:],
                                start=True,
                                stop=True,
                            )
                        nc.vector.tensor_mul(
                            xatt8[64 * side : 64 * side + 64, j // 2, j % 2, s],
                            un_bf[2 * j + side][0:64, s],
                            pinvb[0:64, :],
                        )

            # software pipeline: AV matmuls of iteration j-1 are interleaved
            # two-per-mk between the QK matmuls of iteration j, so the
            # activation engine (the attention bottleneck) never starves and
            # the PE never idles past the HAM window. All V^T tiles are built
            # upfront while the activation engine is still idle (QKV phase).
            for j in range(4):
                build_vt(j)
            for jq in range(5):
                jp = jq - 1
                avl = list(av_plan) if jp >= 0 else []
                if jq < 4:
                    es = e8[jq % 2]
                    for mk in range(8):
                        pq = [pstile([128, NSP], F32), pstile([128, NSP], F32)]
                        for n2 in range(2):
                            s = slice(n2 * 512, (n2 + 1) * 512)
                            for side in range(2):
                                r0 = 64 * side
                                nc.tensor.matmul(
                                    pq[side][:, s],
                                    qk_sb[4 + jq][
                                        r0 : r0 + 64, mk * 128 : (mk + 1) * 128
                                    ],
                                    qk_sb[jq][r0 : r0 + 64, s],
                                    start=True,
                                    stop=True,
                                    tile_position=(r0, 0),
                                )
                        t, i = mk // 2, mk % 2
                        for side in range(2):
                            nc.scalar.activation(
                                es[side][:, t, i, :],
                                pq[side][:],
                                AF.Exp,
                                scale=0.125,
                                bias=bias2[:],
                            )
                        for _ in range(2):
                            if avl:
                                av_step(jp, *avl.pop(0))
                    if jp >= 0:
                        dance(jp)
                else:
                    for st in avl:
                        av_step(jp, *st)
                    dance(jp)

            if KSTAGE == 3:
                dump8([xatt8[:, t // 2, t % 2, :] for t in range(CT)])
                return nc

            # ---- out projection (fp8 DR; keep f32 out for GN2 stats) ----
            attn2 = [
                pers.tile([128, NSP], F32, tag=f"attn2{t}", name=f"attn2{t}")
                for t in range(CT)
            ]
            gn2_r3 = []
            for m in range(CT):
                ps = pstile([128, NSP], F32)
                for n2 in range(2):
                    s = slice(n2 * 512, (n2 + 1) * 512)
                    for t in range(2):
                        nc.tensor.matmul(
                            ps[:, s],
                            wo_sb[:, t, :, m * 128 : (m + 1) * 128],
                            xatt8[:, t, :, s],
                            start=(t == 0),
                            stop=(t == 1),
                            perf_mode=DR,
                        )
                nc.vector.tensor_scalar_add(attn2[m][:], ps[:], outb_sb[:, m : m + 1])
                gn2_r3.append(gn_stats(attn2[m], m))

            if KSTAGE == 4:
                for m in range(CT):
                    nc.sync.dma_start(out_d[m * 128 : (m + 1) * 128, :], attn2[m][:])
                return nc

            # ---- GN2 -> xn8 (reuse) ----
            gn_finish(gn2_r3, attn2, g2_sb, b2_sb, xn8)

            if KSTAGE == 5:
                dump8([xn8[:, t // 2, t % 2, :] for t in range(CT)])
                return nc

            # ---- MLP1 + SwiGLU -> act8 (fp8, DR layout) ----
            act8 = pers.tile([128, 8, 2, NSP], FP8, tag="act8", name="act8")
            for mp in range(16):
                ps1 = pstile([128, NSP], F32)
                for n2 in range(2):
                    s = slice(n2 * 512, (n2 + 1) * 512)
                    for t in range(2):
                        nc.tensor.matmul(
                            ps1[:, s],
                            w1_sb[:, t, :, mp * 128 : (mp + 1) * 128],
                            xn8[:, t, :, s],
                            start=(t == 0),
                            stop=(t == 1),
                            perf_mode=DR,
                        )
                sg = swp.tile([128, NSP], BF16, tag="sw", name="sw")
                for n2 in range(2):
                    s = slice(n2 * 512, (n2 + 1) * 512)
                    ps2h = pstile_av([128, 512], F32, n2)
                    for t in range(2):
                        nc.tensor.matmul(
                            ps2h[:, :],
                            w1_sb[:, t, :, (mp + 16) * 128 : (mp + 17) * 128],
                            xn8[:, t, :, s],
                            start=(t == 0),
                            stop=(t == 1),
                            perf_mode=DR,
                        )
                    nc.scalar.activation(sg[:, s], ps1[:, s], AF.Silu)
                    nc.vector.tensor_mul(
                        act8[:, mp // 2, mp % 2, s], sg[:, s], ps2h[:, :]
                    )

            if KSTAGE == 6:
                dump8([act8[:, t // 2, t % 2, :] for t in range(CT)])
                return nc

            # ---- MLP2 + residual -> out ----
            for m in range(CT):
                ps = pstile([128, NSP], F32)
                for n2 in range(2):
                    s = slice(n2 * 512, (n2 + 1) * 512)
                    for t in range(8):
                        nc.tensor.matmul(
                            ps[:, s],
                            w2_sb[:, t, :, m * 128 : (m + 1) * 128],
                            act8[:, t, :, s],
                            start=(t == 0),
                            stop=(t == 7),
                            perf_mode=DR,
                        )
                nc.vector.tensor_tensor(x_sb[m][:], ps[:], x_sb[m][:], op=ADD)
                nc.sync.dma_start(out_d[m * 128 : (m + 1) * 128, :], x_sb[m][:])

    return nc


def _get_nc():
    key = ("nc", KSTAGE)
    if key not in _cache:
        import concourse.bass  # noqa: F401  ensure importable before build
        from concourse import mybir

        res = _build_nc()
        nc = res[0] if isinstance(res, tuple) else res
        _split_multi_waits(nc, mybir, maxw=1)
        _cache[key] = nc
    return _cache[key]


def _prep_weights(inputs):
    bf = ml_dtypes.bfloat16
    f8 = ml_dtypes.float8_e4m3
    f32 = np.float32

    def col4(v):  # (512,) -> (128, 4) with [p, t] = v[128t + p]
        return np.ascontiguousarray(v.reshape(4, 128).T.astype(f32))

    def dr_lhst(w, pairs):  # w: (O, K) -> [128, pairs, 2, O] fp8 DoubleRow lhsT
        o, k = w.shape
        assert k == pairs * 256
        wt = w.astype(f32).T.reshape(pairs, 2, 128, o).transpose(2, 0, 1, 3)
        return np.ascontiguousarray(wt).astype(f8)

    qkv_b = inputs["qkv_b"].astype(f32)
    sel = np.zeros((C, 32), f32)
    sel[np.arange(C), np.arange(C) // 16] = 1.0
    selbc = np.zeros((16, 1024), f32)
    for r in range(16):
        selbc[r, r * 64 : (r + 1) * 64] = 1.0
    selbc = selbc.astype(bf)

    shared = {
        "wqkv8": dr_lhst(inputs["qkv_w"], 2),
        "qkvb": np.ascontiguousarray(qkv_b.reshape(12, 128).T.astype(f32)),
        "wo8": dr_lhst(inputs["out_w"], 2),
        "outb": col4(inputs["out_b"].astype(f32)),
        "g1": col4(inputs["gn1_gamma"].astype(f32)),
        "b1": col4(inputs["gn1_beta"].astype(f32)),
        "g2": col4(inputs["gn2_gamma"].astype(f32)),
        "b2": col4(inputs["gn2_beta"].astype(f32)),
        "w18": dr_lhst(inputs["mlp1_w"], 2),
        "w28": dr_lhst(inputs["mlp2_w"], 8),
        "sel": sel,
        "selT": np.ascontiguousarray(sel.T),
        "ident": np.eye(128, dtype=f32).astype(bf),
        "selbc": selbc,
    }
    return shared


def kernel(**inputs):
    from concourse.bass_utils import run_bass_kernel_spmd

    nc = _get_nc()
    shared = _prep_weights(inputs)
    x = np.asarray(inputs["x"], dtype=np.float32).reshape(8, C, NSP)
    in_maps = [dict(shared, x=np.ascontiguousarray(x[i])) for i in range(8)]
    res = run_bass_kernel_spmd(nc, in_maps, core_ids=list(range(8))).results
    out = np.stack([res[i]["out"] for i in range(8)], axis=0)
    return out.reshape(8, C, 32, 32).astype(np.float32)


# revision 23
# speedup vs baseline: 1.0634x; 1.0634x over previous
"""Trainium2 Bass kernel for nn_Attention_Block (dense transformer block).

Strategy: pure data-parallel over batch — 8 samples, 8 NeuronCores, one
sample per core, weights replicated, no collectives. Per core everything
stays channels-on-partitions (c x n layout).

v2: fp8(e4m3) DoubleRow matmuls for all dense GEMMs (QKV, out-proj,
SwiGLU MLP1/MLP2) at ~2x PE rate; attention QK as row-tiled concurrent
matmul pairs (d=64 contraction, two heads in disjoint PE row groups);
AV in fp8 DoubleRow with the softmax denominator riding as a ones
column of V^T; exp emitted as fp8 directly from the activation engine.

  GN1 (bn_stats + selector-matmul group reduce) -> QKV (fp8 DR) ->
  per-head attention (row-tiled K^T Q, exp->fp8, fp8-DR AV+denominator,
  transpose/reciprocal/selector-broadcast normalize) -> out-proj (fp8
  DR) -> GN2 -> SwiGLU MLP (fp8 DR) -> +residual.
"""

import os

import numpy as np
import ml_dtypes

KSTAGE = int(os.environ.get("KSTAGE", "7"))

C = 512
NSP = 1024  # 32*32 spatial
CT = 4  # channel tiles of 128
HEADS = 8
D = 64
HID = 2048
EPS = 1e-5

_cache = {}


def _patch_tile_drain(tile, mybir):
    """walrus in this environment accepts very few sync waits per
    instruction; the TileContext tail drain carries one wait per proc of
    the global clock. Split them across preceding SP drains."""
    if getattr(tile.TileContext, "_drain_patched", False):
        return

    def _patched(self, tick_clock, wait_clock):
        nc = self.nc
        spills = [nc.sync.drain() for _ in range(40)]
        drain_inst = nc.sync.drain()
        wait_clock.add_sem_waits(
            drain_inst.ins, tile.ScopedClock({None: tick_clock.global_clock})
        )
        si = drain_inst.ins.sync_info
        waits = list(si.on_wait) if si is not None and si.on_wait else []
        upds = list(si.on_update) if si is not None and si.on_update else []
        if len(waits) > 1:
            *pre, last = waits
            assert len(pre) <= len(spills), "too many drain wait chunks"
            for sp_inst, w in zip(spills, pre):
                sp_inst.ins.sync_info = mybir.SyncInfo(on_wait=[w], on_update=[])
            drain_inst.ins.sync_info = mybir.SyncInfo(on_wait=[last], on_update=upds)
        nc.all_engine_barrier()
        assert self.sems is not None
        popped = nc._tile_sem_poison_stack.pop()
        assert popped is self._sem_poison
        nc.clear_and_free_semaphores(list(self.sems.allocated().values()))
        nc.all_engine_barrier()

    tile.TileContext._drain_and_barrier = _patched
    tile.TileContext._drain_patched = True


def _split_multi_waits(nc, mybir, maxw=1):
    """Hoist extra sync waits onto same-engine EventSemaphore carriers so
    no instruction carries more than `maxw` waits."""
    f = nc.m.functions[0]
    for bb in f.blocks:
        insts = list(bb.instructions)
        need = [
            i
            for i in insts
            if getattr(i, "sync_info", None)
            and i.sync_info.on_wait
            and len(i.sync_info.on_wait) > maxw
        ]
        if not need:
            continue
        carriers = {}
        for inst in need:
            w = list(inst.sync_info.on_wait)
            upds = list(inst.sync_info.on_update) if inst.sync_info.on_update else []
            keep = w[-maxw:]
            extra = w[:-maxw]
            cs = []
            for i in range(0, len(extra), maxw):
                c = mybir.InstEventSemaphore(
                    name=f"I-waitc-{nc.next_id()}", ins=[], outs=[]
                )
                c.engine = inst.engine
                c.sync_info = mybir.SyncInfo(on_wait=extra[i : i + maxw], on_update=[])
                nc.register_instruction(c)
                cs.append(c)
            inst.sync_info = mybir.SyncInfo(on_wait=keep, on_update=upds)
            carriers[inst.name] = cs
        carrier_names = {c.name for cs in carriers.values() for c in cs}
        rebuilt = []
        for inst in list(bb.instructions):
            if inst.name in carrier_names:
                continue
            if inst.name in carriers:
                rebuilt.extend(carriers[inst.name])
            rebuilt.append(inst)
        bb.instructions = rebuilt


def _build_nc():
    import concourse.bass as bass
    import concourse.tile as tile
    from concourse import mybir

    _patch_tile_drain(tile, mybir)

    F32 = mybir.dt.float32
    BF16 = mybir.dt.bfloat16
    FP8 = mybir.dt.float8e4
    ADD = mybir.AluOpType.add
    SUB = mybir.AluOpType.subtract
    MULT = mybir.AluOpType.mult
    AF = mybir.ActivationFunctionType
    DR = mybir.MatmulPerfMode.DoubleRow

    nc = bass.Bass()

    x_d = nc.declare_dram_parameter("x", [C, NSP], F32, isOutput=False)
    wqkv_d = nc.declare_dram_parameter("wqkv8", [128, 2, 2, 3 * C], FP8, isOutput=False)
    qkvb_d = nc.declare_dram_parameter("qkvb", [128, 12], F32, isOutput=False)
    wo_d = nc.declare_dram_parameter("wo8", [128, 2, 2, C], FP8, isOutput=False)
    outb_d = nc.declare_dram_parameter("outb", [128, 4], F32, isOutput=False)
    g1_d = nc.declare_dram_parameter("g1", [128, 4], F32, isOutput=False)
    b1_d = nc.declare_dram_parameter("b1", [128, 4], F32, isOutput=False)
    g2_d = nc.declare_dram_parameter("g2", [128, 4], F32, isOutput=False)
    b2_d = nc.declare_dram_parameter("b2", [128, 4], F32, isOutput=False)
    w1_d = nc.declare_dram_parameter("w18", [128, 2, 2, 2 * HID], FP8, isOutput=False)
    w2_d = nc.declare_dram_parameter("w28", [128, 8, 2, C], FP8, isOutput=False)
    sel_d = nc.declare_dram_parameter("sel", [C, 32], F32, isOutput=False)
    selT_d = nc.declare_dram_parameter("selT", [32, C], F32, isOutput=False)
    id_d = nc.declare_dram_parameter("ident", [128, 128], BF16, isOutput=False)
    id8_d = nc.declare_dram_parameter("id8", [8, 8], F32, isOutput=False)
    selbc32_d = nc.declare_dram_parameter("selbc32", [32, 2048], BF16, isOutput=False)
    out_d = nc.declare_dram_parameter("out", [C, NSP], F32, isOutput=True)

    with tile.TileContext(nc) as tc:
        with (
            tc.tile_pool(name="pers", bufs=1) as pers,
            tc.tile_pool(name="gnp", bufs=2) as gnp,
            tc.tile_pool(name="swp", bufs=2) as swp,
            tc.tile_pool(name="invp", bufs=2) as invp,
            tc.tile_pool(name="ps", bufs=3, space="PSUM") as ps_pool,
            tc.tile_pool(name="psav", bufs=1, space="PSUM") as psav_pool,
        ):
            def pstile(shape, dtype):
                return ps_pool.tile(shape, dtype, tag="ps", name="ps")

            def pstile_av(shape, dtype, side):
                return psav_pool.tile(shape, dtype, tag=f"pav{side}", name="pav")

            # ---- input loads ----
            x_sb = []
            for t in range(CT):
                xt = pers.tile([128, NSP], F32, tag=f"x{t}", name=f"x{t}")
                nc.sync.dma_start(xt[:], x_d[t * 128 : (t + 1) * 128, :])
                x_sb.append(xt)
            sel_sb = []
            for t in range(CT):
                st = pers.tile([128, 32], F32, tag=f"sel{t}", name=f"sel{t}")
                nc.sync.dma_start(st[:], sel_d[t * 128 : (t + 1) * 128, :])
                sel_sb.append(st)
            selT_sb = pers.tile([32, C], F32, tag="selT", name="selT")
            nc.sync.dma_start(selT_sb[:], selT_d[:, :])
            g1_sb = pers.tile([128, 4], F32, tag="g1", name="g1")
            nc.sync.dma_start(g1_sb[:], g1_d[:, :])
            b1_sb = pers.tile([128, 4], F32, tag="b1", name="b1")
            nc.sync.dma_start(b1_sb[:], b1_d[:, :])
            wqkv_sb = pers.tile([128, 2, 2, 3 * C], FP8, tag="wqkv", name="wqkv")
            nc.sync.dma_start(wqkv_sb[:], wqkv_d[:, :, :, :])
            qkvb_sb = pers.tile([128, 12], F32, tag="qkvb", name="qkvb")
            nc.sync.dma_start(qkvb_sb[:], qkvb_d[:, :])
            id_sb = pers.tile([128, 128], BF16, tag="ident", name="ident")
            nc.sync.dma_start(id_sb[:], id_d[:, :])
            id8_sb = pers.tile([8, 8], F32, tag="id8", name="id8")
            nc.sync.dma_start(id8_sb[:], id8_d[:, :])
            selbc32_sb = pers.tile([32, 2048], BF16, tag="selbc32", name="selbc32")
            nc.sync.dma_start(selbc32_sb[:], selbc32_d[:, :])
            wo_sb = pers.tile([128, 2, 2, C], FP8, tag="wo", name="wo")
            nc.sync.dma_start(wo_sb[:], wo_d[:, :, :, :])
            outb_sb = pers.tile([128, 4], F32, tag="outb", name="outb")
            nc.sync.dma_start(outb_sb[:], outb_d[:, :])
            g2_sb = pers.tile([128, 4], F32, tag="g2", name="g2")
            nc.sync.dma_start(g2_sb[:], g2_d[:, :])
            b2_sb = pers.tile([128, 4], F32, tag="b2", name="b2")
            nc.sync.dma_start(b2_sb[:], b2_d[:, :])
            w1_sb = pers.tile([128, 2, 2, 2 * HID], FP8, tag="w1", name="w1")
            nc.sync.dma_start(w1_sb[:], w1_d[:, :, :, :])
            w2_sb = pers.tile([128, 8, 2, C], FP8, tag="w2", name="w2")
            nc.sync.dma_start(w2_sb[:], w2_d[:, :, :, :])

            # PE warm-up: dense dummy matmuls while DMAs land so the HAM
            # clock gate opens (K=8/8) before the QKV GEMMs start.
            warm = pers.tile([128, 512], BF16, tag="warm", name="warm")
            nc.vector.memset(warm[:], 0.5)
            for _ in range(14):
                pw = pstile_av([128, 512], F32, 0)
                nc.tensor.matmul(
                    pw[:], id_sb[:], warm[:], start=True, stop=True
                )

            eps32 = pers.tile([32, 1], F32, tag="eps", name="eps")
            nc.vector.memset(eps32[:], EPS)
            bias2 = pers.tile([128, 1], F32, tag="bias2", name="bias2")
            nc.vector.memset(bias2[:], 2.0)
            ones64 = pers.tile([1, 64], BF16, tag="ones64", name="ones64")
            nc.vector.memset(ones64[:], 1.0)

            # ---- group norm helper (32 groups of 16 channels x 1024) ----
            def gn_stats(src_tile, t):
                stats = gnp.tile([128, 2, 6], F32, tag="gn_stats", name="gn_stats")
                for j2 in range(2):
                    nc.vector.bn_stats(
                        stats[:, j2, :], src_tile[:, j2 * 512 : (j2 + 1) * 512]
                    )
                mv = gnp.tile([128, 2], F32, tag="gn_mv", name="gn_mv")
                nc.vector.bn_aggr(mv[:], stats[:])
                r3 = gnp.tile([128, 3], F32, tag=f"gn_r3_{t}", name=f"gn_r3_{t}")
                nc.vector.tensor_copy(r3[:, 0:2], mv[:])
                nc.vector.tensor_mul(r3[:, 2:3], mv[:, 0:1], mv[:, 0:1])
                return r3

            def gn_finish(rhs3, src_tiles, gam_sb, bet_sb, dst8):
                pg = pstile([32, 3], F32)
                for t in range(CT):
                    nc.tensor.matmul(
                        pg[:], sel_sb[t][:], rhs3[t][:], start=(t == 0), stop=(t == 3)
                    )
                gs = gnp.tile([32, 2], F32, tag="gn_gs", name="gn_gs")
                tmp = gnp.tile([32, 2], F32, tag="gn_tmp", name="gn_tmp")
                pgs = gnp.tile([32, 3], F32, tag="gn_pgs", name="gn_pgs")
                nc.vector.tensor_copy(pgs[:], pg[:])
                # mean_g, E[x^2]_g, var_g, rstd_g
                nc.vector.tensor_scalar_mul(gs[:, 0:1], pgs[:, 0:1], 1.0 / 16)
                nc.vector.tensor_tensor(tmp[:, 0:1], pgs[:, 1:2], pgs[:, 2:3], op=ADD)
                nc.vector.tensor_scalar_mul(tmp[:, 0:1], tmp[:, 0:1], 1.0 / 16)
                nc.vector.tensor_mul(tmp[:, 1:2], gs[:, 0:1], gs[:, 0:1])
                nc.vector.tensor_tensor(tmp[:, 0:1], tmp[:, 0:1], tmp[:, 1:2], op=SUB)
                nc.scalar.activation(
                    tmp[:, 0:1], tmp[:, 0:1], AF.Sqrt, bias=eps32[:]
                )
                nc.vector.reciprocal(gs[:, 1:2], tmp[:, 0:1])
                for t in range(CT):
                    pbc = pstile([128, 2], F32)
                    nc.tensor.matmul(
                        pbc[:],
                        selT_sb[:, t * 128 : (t + 1) * 128],
                        gs[:],
                        start=True,
                        stop=True,
                    )
                    a_t = gnp.tile([128, 1], F32, tag="gn_A", name="gn_A")
                    b_t = gnp.tile([128, 1], F32, tag="gn_B", name="gn_B")
                    nc.vector.tensor_mul(a_t[:], pbc[:, 1:2], gam_sb[:, t : t + 1])
                    nc.vector.tensor_mul(b_t[:], pbc[:, 0:1], a_t[:])
                    nc.vector.tensor_tensor(
                        b_t[:], bet_sb[:, t : t + 1], b_t[:], op=SUB
                    )
                    nc.vector.tensor_scalar(
                        dst8[:, t // 2, t % 2, :],
                        src_tiles[t][:],
                        scalar1=a_t[:],
                        scalar2=b_t[:],
                        op0=MULT,
                        op1=ADD,
                    )

            # ---- GN1 -> xn8 (fp8, DoubleRow rhs layout) ----
            xn8 = pers.tile([128, 2, 2, NSP], FP8, tag="xn8", name="xn8")
            gn_finish([gn_stats(x_sb[t], t) for t in range(CT)],
                      x_sb, g1_sb, b1_sb, xn8)

            def dump8(slices):
                # slices: list of 4 fp8 [128, NSP] APs -> f32 dram
                for t in range(CT):
                    ft = pers.tile([128, NSP], F32, tag=f"dump{t}", name=f"dump{t}")
                    nc.vector.tensor_copy(ft[:], slices[t])
                    nc.sync.dma_start(out_d[t * 128 : (t + 1) * 128, :], ft[:])

            if KSTAGE == 1:
                dump8([xn8[:, t // 2, t % 2, :] for t in range(CT)])
                return nc

            # ---- QKV (12 out tiles of 128 x 1024; q/k bf16, v bf16) ----
            qk_sb = [
                pers.tile([128, NSP], BF16, tag=f"qk{m}", name=f"qk{m}")
                for m in range(8)
            ]
            v_sb = [
                pers.tile([128, NSP], BF16, tag=f"v{m}", name=f"v{m}")
                for m in range(4)
            ]
            for m in range(12):
                ps = pstile([128, NSP], F32)
                for n2 in range(2):
                    s = slice(n2 * 512, (n2 + 1) * 512)
                    for t in range(2):
                        nc.tensor.matmul(
                            ps[:, s],
                            wqkv_sb[:, t, :, m * 128 : (m + 1) * 128],
                            xn8[:, t, :, s],
                            start=(t == 0),
                            stop=(t == 1),
                            perf_mode=DR,
                        )
                dst = qk_sb[m] if m < 8 else v_sb[m - 8]
                nc.scalar.activation(
                    dst[:], ps[:], AF.Identity, bias=qkvb_sb[:, m : m + 1]
                )

            if KSTAGE == 2:
                dump8([qk_sb[t][:] for t in range(CT)])
                return nc

            # ---- attention (head pairs j: even head rows 0:64, odd 64:128) ----
            # vt2 layout per (pair t, sub i): [0:64]=V_even^T, [64]=ones,
            # [80:144]=V_odd^T, [144]=ones  (i-stride 176 B, %16 == 0)
            vts = []
            for vi in range(4):
                vt = pers.tile([128, 4, 2, 176], FP8, tag=f"vt{vi}", name=f"vt{vi}")
                nc.vector.memset(vt[:, :, :, 64:65], 1.0)
                nc.vector.memset(vt[:, :, :, 144:145], 1.0)
                vts.append(vt)
            # e8 buffers per (parity, side)
            e8 = [
                [
                    pers.tile(
                        [128, 4, 2, NSP], FP8, tag=f"e8_{p}{s}", name=f"e8_{p}{s}"
                    )
                    for s in range(2)
                ]
                for p in range(2)
            ]
            un_bf = [
                pers.tile([65, NSP], F32, tag=f"un{r}", name=f"un{r}")
                for r in range(8)
            ]
            dn8 = [
                pers.tile([4, NSP], F32, tag=f"dn8{h}", name=f"dn8{h}")
                for h in range(2)
            ]
            xatt8 = pers.tile([128, 2, 2, NSP], FP8, tag="xatt8", name="xatt8")

            def build_vt(j):
                # V^T via PE transposes (bf16) -> fp8 slices of vt
                vt = vts[j]
                for mk in range(8):
                    pv = pstile([128, 128], BF16)
                    nc.tensor.transpose(
                        pv[:], v_sb[j][:, mk * 128 : (mk + 1) * 128], id_sb[:]
                    )
                    t, i = mk // 2, mk % 2
                    nc.vector.tensor_copy(vt[:, t, i, 0:64], pv[:, 0:64])
                    nc.vector.tensor_copy(vt[:, t, i, 80:144], pv[:, 64:128])

            pav_cur = {}

            def av_step(j, side, nh, t):
                # one AV DoubleRow matmul (denominator rides as ones column);
                # interleaved between QK matmuls of iteration j+1.
                vt = vts[j]
                es = e8[j % 2]
                s = slice(nh * 512, (nh + 1) * 512)
                if t == 0:
                    pav_cur[side] = pstile_av([65, 512], F32, side)
                nc.tensor.matmul(
                    pav_cur[side][:, :],
                    vt[:, t, :, 80 * side : 80 * side + 65],
                    es[side][:, t, :, s],
                    start=(t == 0),
                    stop=(t == 3),
                    perf_mode=DR,
                )
                if t == 3:
                    r = 2 * j + side
                    nc.vector.tensor_copy(un_bf[r][:, s], pav_cur[side][0:65, :])
                    nc.sync.dma_start(
                        dn8[r // 4][r % 4 : r % 4 + 1, s], un_bf[r][64:65, s]
                    )

            av_plan = [
                (side, nh, t) for side in range(2) for nh in range(2)
                for t in range(4)
            ]

            def normalize_half(h):
                # softmax normalize for rows 4h..4h+3: block transposes put
                # the denominator rows on partitions, one cheap [128,32] DVE
                # reciprocal, selector-broadcast, multiply.
                pdt = pstile([128, 32], F32)
                for jj in range(8):
                    nc.tensor.transpose(
                        pdt[:, jj * 4 : (jj + 1) * 4],
                        dn8[h][0:4, jj * 128 : (jj + 1) * 128],
                        id8_sb[0:4, 0:4],
                    )
                inv32 = invp.tile([128, 32], F32, tag="inv32", name="inv32")
                nc.vector.reciprocal(inv32[:], pdt[:])
                inv32b = invp.tile([128, 32], BF16, tag="inv32b", name="inv32b")
                nc.vector.tensor_copy(inv32b[:], inv32[:])
                ptv = pstile([32, 128], BF16)
                nc.tensor.transpose(ptv[:], inv32b[:], id_sb[:])
                pts = invp.tile([32, 128], BF16, tag="pts", name="pts")
                nc.vector.tensor_copy(pts[:], ptv[:])
                for r4 in range(4):
                    r = 4 * h + r4
                    j, side = r // 2, r % 2
                    for nh in range(2):
                        s = slice(nh * 512, (nh + 1) * 512)
                        pinvb = pstile([64, 512], F32)
                        for jj in range(4):
                            q = (nh * 4 + jj) * 4 + r4
                            nc.tensor.matmul(
                                pinvb[:, jj * 128 : (jj + 1) * 128],
                                selbc32_sb[:, q * 64 : (q + 1) * 64],
                                pts[:, :],
                                start=True,
                                stop=True,
                            )
                        nc.vector.tensor_mul(
                            xatt8[64 * side : 64 * side + 64, j // 2, j % 2, s],
                            un_bf[r][0:64, s],
                            pinvb[0:64, :],
                        )

            # software pipeline: AV matmuls of iteration j-1 are interleaved
            # two-per-mk between the QK matmuls of iteration j, so the
            # activation engine (the attention bottleneck) never starves and
            # the PE never idles past the HAM window. V^T builds and the first
            # normalize half-batch also hide inside the exp stream.
            build_vt(0)
            for jq in range(5):
                jp = jq - 1
                avl = list(av_plan) if jp >= 0 else []
                if jq < 4:
                    es = e8[jq % 2]
                    for mk in range(8):
                        pq = [pstile([128, NSP], F32), pstile([128, NSP], F32)]
                        for n2 in range(2):
                            s = slice(n2 * 512, (n2 + 1) * 512)
                            for side in range(2):
                                r0 = 64 * side
                                nc.tensor.matmul(
                                    pq[side][:, s],
                                    qk_sb[4 + jq][
                                        r0 : r0 + 64, mk * 128 : (mk + 1) * 128
                                    ],
                                    qk_sb[jq][r0 : r0 + 64, s],
                                    start=True,
                                    stop=True,
                                    tile_position=(r0, 0),
                                )
                        t, i = mk // 2, mk % 2
                        for side in range(2):
                            nc.scalar.activation(
                                es[side][:, t, i, :],
                                pq[side][:],
                                AF.Exp,
                                scale=0.125,
                                bias=bias2[:],
                            )
                        for _ in range(2):
                            if avl:
                                av_step(jp, *avl.pop(0))
                    if jq + 1 < 4:
                        build_vt(jq + 1)
                else:
                    for st in avl:
                        av_step(jp, *st)
                    normalize_half(0)
                    normalize_half(1)

            if KSTAGE == 3:
                dump8([xatt8[:, t // 2, t % 2, :] for t in range(CT)])
                return nc

            # ---- out projection (fp8 DR; keep f32 out for GN2 stats) ----
            attn2 = [
                pers.tile([128, NSP], F32, tag=f"attn2{t}", name=f"attn2{t}")
                for t in range(CT)
            ]
            gn2_r3 = []
            for m in range(CT):
                ps = pstile([128, NSP], F32)
                for n2 in range(2):
                    s = slice(n2 * 512, (n2 + 1) * 512)
                    for t in range(2):
                        nc.tensor.matmul(
                            ps[:, s],
                            wo_sb[:, t, :, m * 128 : (m + 1) * 128],
                            xatt8[:, t, :, s],
                            start=(t == 0),
                            stop=(t == 1),
                            perf_mode=DR,
                        )
                nc.vector.tensor_scalar_add(attn2[m][:], ps[:], outb_sb[:, m : m + 1])
                gn2_r3.append(gn_stats(attn2[m], m))

            if KSTAGE == 4:
                for m in range(CT):
                    nc.sync.dma_start(out_d[m * 128 : (m + 1) * 128, :], attn2[m][:])
                return nc

            # ---- GN2 -> xn8 (reuse) ----
            gn_finish(gn2_r3, attn2, g2_sb, b2_sb, xn8)

            if KSTAGE == 5:
                dump8([xn8[:, t // 2, t % 2, :] for t in range(CT)])
                return nc

            # ---- MLP1 + SwiGLU -> act8 (fp8, DR layout) ----
            act8 = pers.tile([128, 8, 2, NSP], FP8, tag="act8", name="act8")
            for mp in range(16):
                ps1 = pstile([128, NSP], F32)
                for n2 in range(2):
                    s = slice(n2 * 512, (n2 + 1) * 512)
                    for t in range(2):
                        nc.tensor.matmul(
                            ps1[:, s],
                            w1_sb[:, t, :, mp * 128 : (mp + 1) * 128],
                            xn8[:, t, :, s],
                            start=(t == 0),
                            stop=(t == 1),
                            perf_mode=DR,
                        )
                sg = swp.tile([128, NSP], BF16, tag="sw", name="sw")
                for n2 in range(2):
                    s = slice(n2 * 512, (n2 + 1) * 512)
                    ps2h = pstile_av([128, 512], F32, n2)
                    for t in range(2):
                        nc.tensor.matmul(
                            ps2h[:, :],
                            w1_sb[:, t, :, (mp + 16) * 128 : (mp + 17) * 128],
                            xn8[:, t, :, s],
                            start=(t == 0),
                            stop=(t == 1),
                            perf_mode=DR,
                        )
                    nc.scalar.activation(sg[:, s], ps1[:, s], AF.Silu)
                    nc.vector.tensor_mul(
                        act8[:, mp // 2, mp % 2, s], sg[:, s], ps2h[:, :]
                    )

            if KSTAGE == 6:
                dump8([act8[:, t // 2, t % 2, :] for t in range(CT)])
                return nc

            # ---- MLP2 + residual -> out ----
            for m in range(CT):
                ps = pstile([128, NSP], F32)
                for n2 in range(2):
                    s = slice(n2 * 512, (n2 + 1) * 512)
                    for t in range(8):
                        nc.tensor.matmul(
                            ps[:, s],
                            w2_sb[:, t, :, m * 128 : (m + 1) * 128],
                            act8[:, t, :, s],
                            start=(t == 0),
                            stop=(t == 7),
                            perf_mode=DR,
                        )
                nc.vector.tensor_tensor(x_sb[m][:], ps[:], x_sb[m][:], op=ADD)
                nc.sync.dma_start(out_d[m * 128 : (m + 1) * 128, :], x_sb[m][:])

    return nc


def _get_nc():
    key = ("nc", KSTAGE)
    if key not in _cache:
        import concourse.bass  # noqa: F401  ensure importable before build
        from concourse import mybir

        res = _build_nc()
        nc = res[0] if isinstance(res, tuple) else res
        _split_multi_waits(nc, mybir, maxw=1)
        _cache[key] = nc
    return _cache[key]


def _prep_weights(inputs):
    bf = ml_dtypes.bfloat16
    f8 = ml_dtypes.float8_e4m3
    f32 = np.float32

    def col4(v):  # (512,) -> (128, 4) with [p, t] = v[128t + p]
        return np.ascontiguousarray(v.reshape(4, 128).T.astype(f32))

    def dr_lhst(w, pairs):  # w: (O, K) -> [128, pairs, 2, O] fp8 DoubleRow lhsT
        o, k = w.shape
        assert k == pairs * 256
        wt = w.astype(f32).T.reshape(pairs, 2, 128, o).transpose(2, 0, 1, 3)
        return np.ascontiguousarray(wt).astype(f8)

    qkv_b = inputs["qkv_b"].astype(f32)
    sel = np.zeros((C, 32), f32)
    sel[np.arange(C), np.arange(C) // 16] = 1.0

    shared = {
        "wqkv8": dr_lhst(inputs["qkv_w"], 2),
        "qkvb": np.ascontiguousarray(qkv_b.reshape(12, 128).T.astype(f32)),
        "wo8": dr_lhst(inputs["out_w"], 2),
        "outb": col4(inputs["out_b"].astype(f32)),
        "g1": col4(inputs["gn1_gamma"].astype(f32)),
        "b1": col4(inputs["gn1_beta"].astype(f32)),
        "g2": col4(inputs["gn2_gamma"].astype(f32)),
        "b2": col4(inputs["gn2_beta"].astype(f32)),
        "w18": dr_lhst(inputs["mlp1_w"], 2),
        "w28": dr_lhst(inputs["mlp2_w"], 8),
        "sel": sel,
        "selT": np.ascontiguousarray(sel.T),
        "ident": np.eye(128, dtype=f32).astype(bf),
        "id8": np.eye(8, dtype=f32),
        "selbc32": np.ascontiguousarray(
            np.kron(np.eye(32, dtype=f32), np.ones((1, 64), f32))
        ).astype(bf),
    }
    return shared


def kernel(**inputs):
    from concourse.bass_utils import run_bass_kernel_spmd

    nc = _get_nc()
    shared = _prep_weights(inputs)
    x = np.asarray(inputs["x"], dtype=np.float32).reshape(8, C, NSP)
    in_maps = [dict(shared, x=np.ascontiguousarray(x[i])) for i in range(8)]
    res = run_bass_kernel_spmd(nc, in_maps, core_ids=list(range(8))).results
    out = np.stack([res[i]["out"] for i in range(8)], axis=0)
    return out.reshape(8, C, 32, 32).astype(np.float32)
